# revision 41
# baseline (speedup 1.0000x reference)
"""Trainium2 Bass kernel for nn_AttentionNet (encoder layer + 1-step decoder + pointer).

Sharding: data-parallel over batch, 4 batches per core x 8 cores.
Layout strategy: everything kept transposed ([feature, token], feature on
partitions) so no on-chip transposes are needed anywhere; LN stats are
computed with ones/indicator-matmuls over the partition axis; softmax is
max-free (scores are small) with the mask applied multiplicatively after
exp; softmax denominators come from col-packed ones-matmuls.
"""

import math
import os

import numpy as np

MASKMODE = int(os.environ.get("KMASKMODE", "0"))
KATT = int(os.environ.get("KATT", "0"))

import concourse.bacc as bacc
import concourse.bass as bass
import concourse.tile as tile
from concourse import mybir
from concourse.bass_utils import run_bass_kernel_spmd

F32 = mybir.dt.float32
F32R = mybir.dt.float32r
BF16 = mybir.dt.bfloat16
I32 = mybir.dt.int32
AF = mybir.ActivationFunctionType
OP = mybir.AluOpType

E, H, D, FF = 128, 4, 32, 512
HD = H * D
B, T, Q = 32, 1024, 1
NCORES = 8
BPC = B // NCORES  # batches per core
NTK = T // 128     # tk tiles
TQC = 256          # tq chunk for attention
NTQ = T // TQC

WEIGHT_NAMES = [
    "enc_wq", "enc_wk", "enc_wv", "enc_wo", "enc_ln1_g", "enc_ln1_b",
    "enc_ln2_g", "enc_ln2_b", "enc_ffn_w1", "enc_ffn_b1", "enc_ffn_w2",
    "enc_ffn_b2",
    "dec_wq", "dec_wk", "dec_wv", "dec_wo", "dec_ln1_g", "dec_ln1_b",
    "dec_ln2_g", "dec_ln2_b", "dec_ffn_w1", "dec_ffn_b1", "dec_ffn_w2",
    "dec_ffn_b2",
    "ptr_wq", "ptr_wk",
]


def r(ap):
    """View an f32 AP as float32r for full-speed PE matmuls."""
    return ap.bitcast(F32R)


_EMIT_N = [0]


def _emit(nc, tc, tens, dbg, ctx, phases=5):
    rep = _EMIT_N[0]; _EMIT_N[0] += 1
    sc = 1.0 / math.sqrt(D)

    singles = ctx.enter_context(tc.tile_pool(name="singles", bufs=1))
    # One PSUM pool for the whole kernel: S-tag [128,1024] (2 banks) x3 +
    # acc-tag [128,256] (1 bank) x2 = 8 banks exactly.
    psum = ctx.enter_context(tc.tile_pool(name="psum", bufs=1, space="PSUM"))

    cnt = [0]

    def ps_big():
        cnt[0] += 1
        return psum.tile([128, 1024], F32, tag="S", name=f"psS{cnt[0]}", bufs=3)

    def ps_acc():
        cnt[0] += 1
        return psum.tile([128, TQC], F32, tag="acc", name=f"psA{cnt[0]}", bufs=2)

    # ---- weights to SBUF (f32 loads + bf16 casts) ----
    def load_w(shape, nm, in_ap, out_view=None):
        tl = singles.tile(shape, F32, tag=f"{nm}", name=f"{nm}")
        nc.sync.dma_start(out=tl[:] if out_view is None else out_view(tl), in_=in_ap)
        return tl

    def bfcast(tl, nm):
        tb = singles.tile(list(tl.shape), BF16, tag=f"{nm}b", name=f"{nm}b")
        nc.vector.tensor_copy(tb[:], tl[:])
        return tb

    wq_all, wk_all, wv_all, wo_all = {}, {}, {}, {}
    for pfx in ("enc", "dec"):
        for nm, store in (("wq", wq_all), ("wk", wk_all), ("wv", wv_all)):
            tl = load_w([E, HD], f"{pfx}_{nm}",
                        tens[f"{pfx}_{nm}"].ap().rearrange("h e d -> e h d"),
                        out_view=lambda t: t[:].rearrange("e (h d) -> e h d", h=H))
            store[pfx] = bfcast(tl, f"{pfx}_{nm}")
        tl = load_w([HD, E], f"{pfx}_wo", tens[f"{pfx}_wo"].ap().rearrange("h d e -> (h d) e"))
        wo_all[pfx] = bfcast(tl, f"{pfx}_wo")

    w1 = {}; w2bf = {}; b1t = {}; b2c = {}
    for pfx in ("enc", "dec"):
        tl = load_w([E, FF], f"{pfx}_w1", tens[f"{pfx}_ffn_w1"].ap())
        w1[pfx] = bfcast(tl, f"{pfx}_w1")
        tf = load_w([128, 4 * E], f"{pfx}_w2",
                    tens[f"{pfx}_ffn_w2"].ap().rearrange("(c p) e -> p c e", p=128),
                    out_view=lambda t: t[:].rearrange("p (c e) -> p c e", c=4))
        w2bf[pfx] = bfcast(tf, f"{pfx}_w2")
        b1t[pfx] = load_w([128, 4], f"{pfx}_b1",
                          tens[f"{pfx}_ffn_b1"].ap().rearrange("(c p) -> p c", p=128))
        b2c[pfx] = load_w([128, 1], f"{pfx}_b2",
                          tens[f"{pfx}_ffn_b2"].ap().rearrange("(e q) -> e q", q=1))

    ptrq = load_w([E, E], "ptr_wq", tens["ptr_wq"].ap())
    ptrk = load_w([E, E], "ptr_wk", tens["ptr_wk"].ap())

    # LN gain/bias rows, each its own [1, E] tile at partition 0 (f32 + bf16)
    ln_rows = {}
    for nm in ["enc_ln1_g", "enc_ln1_b", "enc_ln2_g", "enc_ln2_b",
               "dec_ln1_g", "dec_ln1_b", "dec_ln2_g", "dec_ln2_b"]:
        row = load_w([1, E], f"row_{nm}", tens[nm].ap().rearrange("(q e) -> q e", q=1))
        ln_rows[nm] = bfcast(row, f"row_{nm}")

    # ---- constants (inline numpy, DMA to SBUF) ----
    def const_tile(arr, dt, nm):
        arr = np.asarray(arr)
        if dt == BF16:
            import ml_dtypes
            arr = arr.astype(ml_dtypes.bfloat16)
        elif dt == F32:
            arr = arr.astype(np.float32)
        else:
            arr = arr.astype(np.int32)
        h = nc.inline_tensor(arr, name=f"c{rep}_{nm}")
        tl = singles.tile(list(arr.shape), dt, tag=f"c_{nm}", name=f"ct_{nm}")
        nc.sync.dma_start(out=tl[:], in_=h.ap())
        return tl

    ones_col_bf = const_tile(np.ones((128, 1)), BF16, "ones_col_bf")
    ones_row = const_tile(np.ones((1, 512)), BF16, "ones_row")
    vc = np.ones((1, 128)); vc[0, ::32] = 0.0
    vcomp_row = const_tile(vc, BF16, "vcomp_row")
    a = np.zeros((128, 128))
    for h in range(H):
        a[32 * h, 32 * h:32 * (h + 1)] = 1.0
    e4sel = const_tile(a, F32, "e4sel")
    # ind8: 8 blocks of width 36: x-stats of batch b -> row b; sq-stats -> row 32+b
    a = np.zeros((128, 8 * 36))
    for b in range(4):
        a[:, 36 * b + b] = 1.0
        a[:, 36 * (4 + b) + 32 + b] = 1.0
    ind8 = const_tile(a, BF16, "ind8")
    a = np.zeros((32, 4))
    for i in range(32):
        a[i, i % 4] = 1.0
    p32 = const_tile(a, F32, "p32")
    a = np.zeros((4, 128))
    for h in range(H):
        a[h, 32 * h:32 * (h + 1)] = 1.0
    e4t = const_tile(a, F32, "e4t")
    eps4 = const_tile(np.full((4, 1), 1e-5), F32, "eps4")
    eps1 = const_tile(np.full((1, 1), 1e-5), F32, "eps1")
    # dsel[k, 4b+h] = 1 iff k == b
    a = np.zeros((4, 16))
    for b in range(BPC):
        a[b, 4 * b:4 * (b + 1)] = 1.0
    dsel = const_tile(a, BF16, "dsel")
    # ind2: [:, 0:33] col0 ones (sum -> row 0); [:, 33:66] col32 ones (sumsq -> row 32)
    a = np.zeros((128, 66))
    a[:, 0] = 1.0
    a[:, 33 + 32] = 1.0
    ind2 = const_tile(a, BF16, "ind2")
    # eb4[0, 4b:4b+4] has 1 at col b (for building gsel rows via K=1 matmuls)
    a = np.zeros((1, 16))
    for b in range(BPC):
        a[0, 4 * b + b] = 1.0
    eb4 = const_tile(a, BF16, "eb4")

    big = ctx.enter_context(tc.tile_pool(name="big", bufs=1))
    scratch = ctx.enter_context(tc.tile_pool(name="scratch", bufs=2))
    sc1 = ctx.enter_context(tc.tile_pool(name="sc1", bufs=1))

    # ---- load srcT ----
    srcT = []
    for b in range(BPC):
        tl = big.tile([E, T], F32, tag=f"srcT{b}")
        nc.sync.dma_start(out=tl[:], in_=tens["src"].ap()[b].rearrange("t e -> e t"))
        srcT.append(tl)

    def ln_group(xs, g_row, b_row):
        """LayerNorm over the partition axis (E) for BPC transposed [128, T] f32 tiles.

        Stats via indicator-matmuls (bf16 inputs), apply via outer-product
        broadcast matmuls. Returns apply(b, out_tile) emitting bf16 output."""
        stats_ps = ps_big()  # rows 0..3 sums, 32..35 sumsq
        nb = len(xs)
        for b, xt in enumerate(xs):
            xbf = sc1.tile([128, T], BF16, tag="lnxbf", name=f"lnxbf{b}", bufs=2)
            nc.vector.tensor_copy(xbf[:], xt[:])
            sq = sc1.tile([128, T], BF16, tag="lnsq", name=f"lnsq{b}", bufs=2)
            nc.gpsimd.tensor_mul(sq[:], xbf[:], xbf[:])
            for c in range(2):
                s = slice(512 * c, 512 * (c + 1))
                nc.tensor.matmul(stats_ps[0:36, s], ind8[:, 36 * b:36 * (b + 1)], xbf[:, s],
                                 start=(b == 0), stop=False)
                nc.tensor.matmul(stats_ps[0:36, s], ind8[:, 36 * (4 + b):36 * (5 + b)], sq[:, s],
                                 start=False, stop=(b == nb - 1))
        stats = scratch.tile([36, T], F32, tag="lnst", bufs=1)
        nc.vector.tensor_copy(stats[:], stats_ps[0:36, :])
        m = scratch.tile([4, T], F32, tag="lnm", bufs=1)
        nc.vector.tensor_scalar_mul(m[0:nb], stats[0:nb], 1.0 / E)
        var = scratch.tile([4, T], F32, tag="lnv", bufs=1)
        nc.vector.tensor_scalar_mul(var[0:nb], stats[32:32 + nb], 1.0 / E)
        msq = scratch.tile([4, T], F32, tag="lnmsq", bufs=1)
        nc.vector.tensor_mul(msq[0:nb], m[0:nb], m[0:nb])
        nc.vector.tensor_sub(var[0:nb], var[0:nb], msq[0:nb])
        nc.scalar.activation(out=var[0:nb], in_=var[0:nb], func=AF.Ln, bias=eps4[0:nb, 0:1])
        rs = scratch.tile([4, T], BF16, tag="lnrs", bufs=1)
        nc.scalar.activation(out=rs[0:nb], in_=var[0:nb], func=AF.Exp, scale=-0.5)
        nmrs = scratch.tile([4, T], BF16, tag="lnnm", bufs=1)
        nc.vector.scalar_tensor_tensor(out=nmrs[0:nb], in0=m[0:nb], scalar=-1.0,
                                       in1=rs[0:nb], op0=OP.mult, op1=OP.mult)
        # gsel[0:4, 128b:128(b+1)] has g in row b, zeros elsewhere
        gsel_ps = ps_big()
        for b in range(nb):
            nc.tensor.matmul(gsel_ps[0:4, E * b:E * (b + 1)], eb4[:, 4 * b:4 * (b + 1)],
                             g_row[:], start=True, stop=True)
        gsel = scratch.tile([4, 4 * E], BF16, tag="gsel", bufs=1)
        nc.vector.tensor_copy(gsel[:], gsel_ps[0:4, 0:4 * E])

        def apply(b, ot):
            xt = xs[b]
            for c in range(2):
                s = slice(512 * c, 512 * (c + 1))
                a_ps = ps_big()
                nc.tensor.matmul(a_ps[:, :512], gsel[:, E * b:E * (b + 1)], rs[0:4, s],
                                 start=True, stop=True)
                b_ps = ps_big()
                nc.tensor.matmul(b_ps[:, :512], gsel[:, E * b:E * (b + 1)], nmrs[0:4, s],
                                 start=True, stop=False)
                nc.tensor.matmul(b_ps[:, :512], b_row[:], ones_row[:], start=False, stop=True)
                tmp = scratch.tile([128, 512], F32, tag="lntmp")
                nc.vector.scalar_tensor_tensor(out=tmp[:], in0=xt[:, s], scalar=1.0,
                                               in1=a_ps[:, :512], op0=OP.bypass, op1=OP.mult)
                nc.vector.scalar_tensor_tensor(out=ot[:, s], in0=tmp[:], scalar=1.0,
                                               in1=b_ps[:, :512], op0=OP.bypass, op1=OP.add)
            return ot
        return apply

    def mk_ln_out(nm):
        return scratch.tile([E, T], BF16, tag="xln", name=nm, bufs=1)

    # ---- encoder LN1 ----
    with nc.named_scope("enc_ln1"):
        apply_ln1 = ln_group(srcT, ln_rows["enc_ln1_g"], ln_rows["enc_ln1_b"])

    # ---- per-batch: QKV + attention ----
    h1T = []
    with nc.named_scope("enc_attn"):
        for b in range(BPC):
            xt = apply_ln1(b, mk_ln_out(f"xln{b}"))
            qhat = scratch.tile([HD, H * T], BF16, tag="qhat")
            nc.vector.memset(qhat[:], 0.0)
            kt = scratch.tile([HD, T], BF16, tag="KT")
            vn = scratch.tile([128, NTK * HD], BF16, tag="Vn")
            for c in range(2):
                s = slice(512 * c, 512 * (c + 1))
                ps = ps_big()
                nc.tensor.matmul(ps[:, :512], wq_all["enc"][:], xt[:, s], start=True, stop=True)
                for h in range(H):
                    dst = qhat[32 * h:32 * (h + 1), :].rearrange(
                        "p (c2 h2 t) -> p c2 h2 t", c2=NTQ, h2=H)[:, 2 * c:2 * c + 2, h, :]
                    src = ps[32 * h:32 * (h + 1), 0:512].rearrange("p (c2 t) -> p c2 t", c2=2)
                    nc.vector.tensor_copy(dst, src)
                ps2 = ps_big()
                nc.tensor.matmul(ps2[:, :512], wk_all["enc"][:], xt[:, s], start=True, stop=True)
                nc.vector.tensor_copy(kt[:, s], ps2[:, :512])
            vps = ps_big()
            for c in range(NTK):
                nc.tensor.matmul(vps[:, HD * c:HD * (c + 1)],
                                 xt[:, 128 * c:128 * (c + 1)], wv_all["enc"][:],
                                 start=True, stop=True)
            nc.vector.tensor_copy(vn[:], vps[:])
            if dbg is not None:
                nc.sync.dma_start(out=dbg["xlnT"].ap()[b], in_=xt[:])

            if phases < 2:
                nc.gpsimd.dma_start(out=tens["out"].ap().rearrange("b q t -> (b q) t"),
                                    in_=qt[0:BPC, :])
                return
            h1 = big.tile([E, T], F32, tag=f"h1T{b}")
            wt = []
            for k in range(NTK):
                mi = scratch.tile([128, T], I32, tag="mi")
                nc.sync.dma_start(
                    out=mi[:],
                    in_=tens["enc_mask"].ap()[b].rearrange("t k -> k t")[128 * k:128 * (k + 1), :])
                w = scratch.tile([128, T], BF16, tag=f"w{k}", bufs=1)
                nc.vector.tensor_scalar(out=w[:], in0=mi[:], scalar1=-1.0, scalar2=1.0,
                                        op0=OP.mult, op1=OP.add)
                wt.append(w)
            if KATT == 1:
                for k in range(NTK):
                    nc.vector.tensor_copy(h1[:, 128 * k:128 * (k + 1)],
                                          wt[k][:, 0:128])
                h1T.append(h1)
                continue
            for tq in range(NTQ):
                qs = slice(TQC * tq, TQC * (tq + 1))
                vacc = ps_acc()
                dacc = ps_acc()
                nc.tensor.matmul(dacc[:], vcomp_row[:], ones_row[:, :TQC], start=True, stop=False)
                for k in range(NTK):
                    S = ps_big()
                    for half in range(2):
                        fs = slice(512 * half, 512 * (half + 1))
                        nc.tensor.matmul(S[:, fs], kt[:, 128 * k:128 * (k + 1)],
                                         qhat[:, H * TQC * tq + 512 * half:
                                              H * TQC * tq + 512 * (half + 1)],
                                         start=True, stop=True)
                    eu = scratch.tile([128, H * TQC], BF16, tag="eu")
                    nc.scalar.activation(out=eu[:], in_=S[:], func=AF.Exp, scale=sc)
                    ws = wt[k][:, qs]
                    if MASKMODE == 0:
                        w_rep = bass.AP(tensor=ws.tensor, offset=ws.offset,
                                        ap=[ws.ap[0], [0, H], ws.ap[1]])
                        eu3 = eu[:].rearrange("p (h t) -> p h t", h=H)
                        nc.gpsimd.tensor_tensor(out=eu3, in0=eu3, in1=w_rep, op=OP.mult)
                    elif MASKMODE == 1:
                        for h in range(H):
                            es0 = eu[:, TQC * h:TQC * (h + 1)]
                            nc.vector.tensor_mul(es0, es0, ws)
                    else:
                        for h in range(H):
                            es0 = eu[:, TQC * h:TQC * (h + 1)]
                            nc.gpsimd.tensor_mul(es0, es0, ws)
                    for h in range(H):
                        es = eu[:, TQC * h:TQC * (h + 1)]
                        nc.tensor.matmul(vacc[32 * h:32 * (h + 1), :],
                                         vn[:, HD * k + 32 * h:HD * k + 32 * (h + 1)],
                                         es, start=(k == 0), stop=(k == NTK - 1),
                                         tile_position=(0, 32 * h))
                        nc.tensor.matmul(dacc[32 * h:32 * h + 1, :],
                                         ones_col_bf[:], es,
                                         start=False, stop=(k == NTK - 1 and h == H - 1),
                                         tile_position=(0, 32 * h))
                recip = scratch.tile([128, TQC], F32, tag="recip")
                nc.vector.reciprocal(recip[:], dacc[:])
                rb_ps = ps_acc()
                nc.tensor.matmul(rb_ps[:], e4sel[:], recip[:], start=True, stop=True)
                rb = scratch.tile([128, TQC], F32, tag="rbs")
                nc.vector.tensor_copy(rb[:], rb_ps[:])
                hn = scratch.tile([128, TQC], BF16, tag="hn")
                nc.vector.tensor_mul(hn[:], vacc[:], rb[:])
                at_ps = ps_acc()
                nc.tensor.matmul(at_ps[:], wo_all["enc"][:], hn[:], start=True, stop=True)
                nc.vector.tensor_add(h1[:, qs], at_ps[:], srcT[b][:, qs])
                if dbg is not None:
                    nc.sync.dma_start(out=dbg["headsT"].ap()[b][:, qs], in_=hn[:])
            if KATT == 2:
                h1T.append(h1)
                continue
            h1T.append(h1)
            if dbg is not None:
                nc.sync.dma_start(out=dbg["h1T"].ap()[b], in_=h1[:])

    if phases < 3:
        nc.sync.dma_start(out=tens["out"].ap().rearrange("b q t -> (b q) t"),
                          in_=h1T[0][0:BPC, :])
        return

    # ---- encoder LN2 + FFN ----
    with nc.named_scope("enc_ffn"):
        apply_ln2 = ln_group(h1T, ln_rows["enc_ln2_g"], ln_rows["enc_ln2_b"])
        memT = []
        for b in range(BPC):
            hlnb = apply_ln2(b, mk_ln_out(f"hln{b}"))
            mt = big.tile([E, T], F32, tag=f"memT{b}")
            act1 = scratch.tile([128, 4 * T], BF16, tag="act1", bufs=1)
            for fc in range(4):
                for c in range(2):
                    s = slice(512 * c, 512 * (c + 1))
                    ps = ps_big()
                    nc.tensor.matmul(ps[:, :512], w1["enc"][:, 128 * fc:128 * (fc + 1)],
                                     hlnb[:, s], start=True, stop=True)
                    nc.scalar.activation(out=act1[:, T * fc + 512 * c:T * fc + 512 * (c + 1)],
                                         in_=ps[:, :512], func=AF.Relu,
                                         bias=b1t["enc"][:, fc:fc + 1], scale=1.0)
            for c in range(2):
                s = slice(512 * c, 512 * (c + 1))
                ps = ps_big()
                for fc in range(4):
                    nc.tensor.matmul(ps[:, :512], w2bf["enc"][:, 128 * fc:128 * (fc + 1)],
                                     act1[:, T * fc + 512 * c:T * fc + 512 * (c + 1)],
                                     start=(fc == 0), stop=(fc == 3))
                nc.vector.scalar_tensor_tensor(out=mt[:, s], in0=ps[:, :512],
                                               scalar=b2c["enc"][:, 0:1],
                                               in1=h1T[b][:, s], op0=OP.add, op1=OP.add)
            memT.append(mt)
            if dbg is not None:
                nc.sync.dma_start(out=dbg["memT"].ap()[b], in_=mt[:])

    if phases < 4:
        nc.sync.dma_start(out=tens["out"].ap().rearrange("b q t -> (b q) t"),
                          in_=memT[0][0:BPC, :])
        return

    # ---- decoder ----
    with nc.named_scope("decoder"):
        tgtT = singles.tile([E, BPC], F32, tag="tgtT")
        nc.sync.dma_start(out=tgtT[:], in_=tens["tgt"].ap().rearrange("b q e -> e (b q)"))
        dmi = singles.tile([BPC, T], I32, tag="dmi")
        nc.sync.dma_start(out=dmi[:], in_=tens["dec_mask"].ap().rearrange("b q t -> (b q) t"))
        dmf = singles.tile([BPC, T], BF16, tag="dmf")  # -10000 * mask (decoder attn)
        nc.vector.tensor_scalar_mul(dmf[:], dmi[:], -10000.0)
        dmf32 = singles.tile([BPC, T], F32, tag="dmf32")  # exact, for pointer logits
        nc.vector.tensor_scalar_mul(dmf32[:], dmi[:], -10000.0)
        dwf = singles.tile([BPC, T], F32, tag="dwf")  # 1 - mask
        nc.vector.tensor_scalar(out=dwf[:], in0=dmi[:], scalar1=-1.0, scalar2=1.0,
                                op0=OP.mult, op1=OP.add)

        apply_lnm = ln_group(memT, ln_rows["dec_ln1_g"], ln_rows["dec_ln1_b"])

        def ln_small(x, n, g_row, b_row, tagn):
            """LN over partitions for a small [E, n] f32 tile -> bf16 out."""
            xbf = scratch.tile([E, BPC], BF16, tag=f"dxb{tagn}")
            nc.vector.tensor_copy(xbf[:, :n], x[:, :n])
            sq = scratch.tile([E, BPC], BF16, tag=f"dsq{tagn}")
            nc.gpsimd.tensor_mul(sq[:, :n], xbf[:, :n], xbf[:, :n])
            st_ps = ps_big()  # row 0 (sum), row 32 (sumsq) via ind2
            nc.tensor.matmul(st_ps[0:33, :n], ind2[:, 0:33], xbf[:, :n], start=True, stop=False)
            nc.tensor.matmul(st_ps[0:33, :n], ind2[:, 33:66], sq[:, :n], start=False, stop=True)
            st = scratch.tile([33, BPC], F32, tag=f"dss{tagn}")
            nc.vector.tensor_copy(st[:, :n], st_ps[0:33, :n])
            mn = scratch.tile([1, BPC], F32, tag=f"dmn{tagn}")
            nc.vector.tensor_scalar_mul(mn[:, :n], st[0:1, :n], 1.0 / E)
            msq = scratch.tile([1, BPC], F32, tag=f"dmsq{tagn}")
            nc.vector.tensor_mul(msq[:, :n], mn[:, :n], mn[:, :n])
            var = scratch.tile([1, BPC], F32, tag=f"dvar{tagn}")
            nc.vector.tensor_scalar(out=var[:, :n], in0=st[32:33, :n], scalar1=1.0 / E,
                                    scalar2=None, op0=OP.mult)
            nc.vector.tensor_sub(var[:, :n], var[:, :n], msq[:, :n])
            nc.scalar.activation(out=var[:, :n], in_=var[:, :n], func=AF.Ln, bias=eps1[:])
            rs = scratch.tile([1, BPC], BF16, tag=f"drs{tagn}")
            nc.scalar.activation(out=rs[:, :n], in_=var[:, :n], func=AF.Exp, scale=-0.5)
            nmrs = scratch.tile([1, BPC], BF16, tag=f"dnm{tagn}")
            nc.vector.scalar_tensor_tensor(out=nmrs[:, :n], in0=mn[:, :n], scalar=-1.0,
                                           in1=rs[:, :n], op0=OP.mult, op1=OP.mult)
            a_ps = ps_acc()
            nc.tensor.matmul(a_ps[:, :n], g_row[:], rs[:, :n], start=True, stop=True)
            b_ps = ps_acc()
            nc.tensor.matmul(b_ps[:, :n], g_row[:], nmrs[:, :n], start=True, stop=False)
            nc.tensor.matmul(b_ps[:, :n], b_row[:], ones_row[:, :n], start=False, stop=True)
            tmp = scratch.tile([E, BPC], F32, tag=f"dtmp{tagn}")
            nc.vector.scalar_tensor_tensor(out=tmp[:, :n], in0=x[:, :n], scalar=1.0,
                                           in1=a_ps[:, :n], op0=OP.bypass, op1=OP.mult)
            out = scratch.tile([E, BPC], BF16, tag=f"dout{tagn}")
            nc.vector.scalar_tensor_tensor(out=out[:, :n], in0=tmp[:, :n], scalar=1.0,
                                           in1=b_ps[:, :n], op0=OP.bypass, op1=OP.add)
            return out

        tln = ln_small(tgtT, BPC, ln_rows["dec_ln1_g"], ln_rows["dec_ln1_b"], "t")
        qd_ps = ps_acc()
        nc.tensor.matmul(qd_ps[0:HD, 0:BPC], wq_all["dec"][:], tln[:, :BPC], start=True, stop=True)
        qdec = scratch.tile([HD, BPC], BF16, tag="qdec")
        nc.vector.tensor_copy(qdec[:], qd_ps[0:HD, 0:BPC])

        h1d = singles.tile([E, BPC], F32, tag="h1d")
        for b in range(BPC):
            mlnb = apply_lnm(b, mk_ln_out(f"mln{b}"))
            kd = scratch.tile([HD, T], BF16, tag="kdec", bufs=1)
            vd = scratch.tile([128, NTK * HD], BF16, tag="vdec", bufs=1)
            for c in range(2):
                s = slice(512 * c, 512 * (c + 1))
                ps = ps_big()
                nc.tensor.matmul(ps[:, :512], wk_all["dec"][:], mlnb[:, s], start=True, stop=True)
                nc.vector.tensor_copy(kd[:, s], ps[:, :512])
            vps = ps_big()
            for c in range(NTK):
                nc.tensor.matmul(vps[:, HD * c:HD * (c + 1)],
                                 mlnb[:, 128 * c:128 * (c + 1)], wv_all["dec"][:],
                                 start=True, stop=True)
            nc.vector.tensor_copy(vd[:], vps[:])
            qblk = scratch.tile([HD, 4], BF16, tag="qblk")
            nc.vector.memset(qblk[:], 0.0)
            for h in range(H):
                nc.vector.tensor_copy(qblk[32 * h:32 * (h + 1), h:h + 1],
                                      qdec[32 * h:32 * (h + 1), b:b + 1])
            ud_ps = ps_acc()
            for k in range(NTK):
                cs = slice(4 * k, 4 * (k + 1))
                nc.tensor.matmul(ud_ps[:, cs], kd[:, 128 * k:128 * (k + 1)], qblk[:],
                                 start=True, stop=False)
                nc.tensor.matmul(ud_ps[:, cs], dmf[0:4, 128 * k:128 * (k + 1)],
                                 dsel[:, 4 * b:4 * (b + 1)], start=False, stop=True)
            eud = scratch.tile([128, 4 * NTK], BF16, tag="eud")
            nc.scalar.activation(out=eud[:], in_=ud_ps[:, 0:4 * NTK], func=AF.Exp, scale=sc)
            d1_ps = ps_acc()
            nc.tensor.matmul(d1_ps[0:32, 0:1], eud[:], ones_col_bf[:], start=True, stop=True)
            d1 = scratch.tile([32, 1], F32, tag="d1s")
            nc.vector.tensor_copy(d1[:], d1_ps[0:32, 0:1])
            d4_ps = ps_acc()
            nc.tensor.matmul(d4_ps[0:4, 0:1], p32[:], d1[:], start=True, stop=True)
            rc4 = scratch.tile([4, 1], F32, tag="rc4")
            nc.vector.reciprocal(rc4[:], d4_ps[0:4, 0:1])
            rb_ps = ps_acc()
            nc.tensor.matmul(rb_ps[:, 0:1], e4t[:], rc4[:], start=True, stop=True)
            rb = scratch.tile([128, 1], F32, tag="rb128s")
            nc.vector.tensor_copy(rb[:], rb_ps[:, 0:1])
            hd_ps = ps_acc()
            for k in range(NTK):
                nc.tensor.matmul(hd_ps[:, 0:4], vd[:, HD * k:HD * (k + 1)], eud[:, 4 * k:4 * (k + 1)],
                                 start=(k == 0), stop=(k == NTK - 1))
            hdec = scratch.tile([HD, 1], BF16, tag="hdec")
            for h in range(H):
                nc.vector.tensor_copy(hdec[32 * h:32 * (h + 1), 0:1],
                                      hd_ps[32 * h:32 * (h + 1), h:h + 1])
            nc.vector.tensor_scalar_mul(hdec[:], hdec[:], rb[:, 0:1])
            ao_ps = ps_acc()
            nc.tensor.matmul(ao_ps[:, 0:1], wo_all["dec"][:], hdec[:], start=True, stop=True)
            nc.vector.tensor_add(h1d[:, b:b + 1], ao_ps[:, 0:1], tgtT[:, b:b + 1])

        hln2d = ln_small(h1d, BPC, ln_rows["dec_ln2_g"], ln_rows["dec_ln2_b"], "d2")
        dact_ps = ps_acc()
        for fc in range(4):
            nc.tensor.matmul(dact_ps[:, 4 * fc:4 * (fc + 1)],
                             w1["dec"][:, 128 * fc:128 * (fc + 1)], hln2d[:, :BPC],
                             start=True, stop=True)
        dact = scratch.tile([128, 16], BF16, tag="dacts")
        for fc in range(4):
            nc.scalar.activation(out=dact[:, 4 * fc:4 * (fc + 1)],
                                 in_=dact_ps[:, 4 * fc:4 * (fc + 1)], func=AF.Relu,
                                 bias=b1t["dec"][:, fc:fc + 1], scale=1.0)
        do_ps = ps_acc()
        for fc in range(4):
            nc.tensor.matmul(do_ps[:, 0:BPC], w2bf["dec"][:, 128 * fc:128 * (fc + 1)],
                             dact[:, 4 * fc:4 * (fc + 1)],
                             start=(fc == 0), stop=(fc == 3))
        decT = singles.tile([E, BPC], F32, tag="decT")
        tmp2 = scratch.tile([E, BPC], F32, tag="dtmp2")
        nc.vector.tensor_scalar_add(tmp2[:], do_ps[:, 0:BPC], b2c["dec"][:, 0:1])
        nc.vector.tensor_add(decT[:], tmp2[:], h1d[:])
        if dbg is not None:
            nc.sync.dma_start(out=dbg["decT"].ap(), in_=decT[:])

    if phases < 5:
        nc.sync.dma_start(out=tens["out"].ap().rearrange("b q t -> (b q) t"),
                          in_=h1d[0:BPC, 0:T] if False else memT[0][0:BPC, :])
        return

    # ---- pointer (f32 matmuls for accuracy) ----
    with nc.named_scope("pointer"):
        qp_ps = ps_acc()
        nc.tensor.matmul(qp_ps[:, 0:BPC], ptrq[:], decT[:], start=True, stop=True)
        qpi = scratch.tile([E, 4 * BPC], F32, tag="qpi")
        nc.vector.memset(qpi[:], 0.0)
        for b in range(BPC):
            nc.vector.tensor_copy(qpi[:, 5 * b:5 * b + 1], qp_ps[:, b:b + 1])
        up_ps = ps_big()
        for b in range(BPC):
            kp = scratch.tile([E, T], F32, tag="kdec", name="kps", bufs=1)
            for c in range(2):
                s = slice(512 * c, 512 * (c + 1))
                kpc = ps_big()
                nc.tensor.matmul(kpc[:, :512], ptrk[:], memT[b][:, s], start=True, stop=True)
                nc.vector.tensor_copy(kp[:, s], kpc[:, :512])
            for c in range(2):
                s = slice(512 * c, 512 * (c + 1))
                nc.tensor.matmul(up_ps[0:BPC, s], qpi[:, 4 * b:4 * (b + 1)], kp[:, s],
                                 start=(b == 0), stop=(b == BPC - 1))
        # L = 10*tanh(U/sqrt(E)) ; tanh(x) = 1 - 2/(exp(2x)+1)
        e2 = scratch.tile([BPC, T], F32, tag="lnm", name="pe2", bufs=1)
        nc.scalar.activation(out=e2[:], in_=up_ps[0:BPC, :], func=AF.Exp, scale=2.0 / math.sqrt(E))
        nc.vector.tensor_scalar_add(e2[:], e2[:], 1.0)
        rec = scratch.tile([BPC, T], F32, tag="lnv", name="prec", bufs=1)
        nc.vector.reciprocal(rec[:], e2[:])
        L = scratch.tile([BPC, T], F32, tag="lnmsq", name="pL", bufs=1)
        nc.vector.tensor_scalar(out=L[:], in0=rec[:], scalar1=-20.0, scalar2=10.0,
                                op0=OP.mult, op1=OP.add)
        if dbg is not None:
            nc.sync.dma_start(out=dbg["Lraw"].ap(), in_=L[:])
        nc.vector.tensor_mul(L[:], L[:], dwf[:])
        nc.vector.tensor_add(L[:], L[:], dmf32[:])
        et = scratch.tile([BPC, T], F32, tag="lnrs2", name="pet", bufs=1)
        se = scratch.tile([BPC, 1], F32, tag="se")
        nc.scalar.activation(out=et[:], in_=L[:], func=AF.Exp, accum_out=se[:])
        lse = scratch.tile([BPC, 1], F32, tag="lse")
        nc.scalar.activation(out=lse[:], in_=se[:], func=AF.Ln)
        res = scratch.tile([BPC, T], F32, tag="lnnm2", name="pres", bufs=1)
        nc.vector.tensor_scalar(out=res[:], in0=L[:], scalar1=lse[:, 0:1], scalar2=None,
                                op0=OP.subtract)
        nc.sync.dma_start(out=tens["out"].ap().rearrange("b q t -> (b q) t"), in_=res[:])


def build(debug=False):
    import contextlib
    nc = bacc.Bacc()
    tens = {}
    tens["src"] = nc.dram_tensor("src", [BPC, T, E], F32, kind="ExternalInput")
    tens["tgt"] = nc.dram_tensor("tgt", [BPC, Q, E], F32, kind="ExternalInput")
    tens["enc_mask"] = nc.dram_tensor("enc_mask", [BPC, T, T], I32, kind="ExternalInput")
    tens["dec_mask"] = nc.dram_tensor("dec_mask", [BPC, Q, T], I32, kind="ExternalInput")
    shapes = {
        "wq": [H, E, D], "wk": [H, E, D], "wv": [H, E, D], "wo": [H, D, E],
        "ln1_g": [E], "ln1_b": [E], "ln2_g": [E], "ln2_b": [E],
        "ffn_w1": [E, FF], "ffn_b1": [FF], "ffn_w2": [FF, E], "ffn_b2": [E],
    }
    for pfx in ("enc", "dec"):
        for nm, shp in shapes.items():
            full = f"{pfx}_{nm}"
            tens[full] = nc.dram_tensor(full, shp, F32, kind="ExternalInput")
    tens["ptr_wq"] = nc.dram_tensor("ptr_wq", [E, E], F32, kind="ExternalInput")
    tens["ptr_wk"] = nc.dram_tensor("ptr_wk", [E, E], F32, kind="ExternalInput")
    tens["out"] = nc.dram_tensor("out", [BPC, Q, T], F32, kind="ExternalOutput")

    dbg = None
    if debug:
        dbg = {
            "xlnT": nc.dram_tensor("dbg_xlnT", [BPC, E, T], BF16, kind="ExternalOutput"),
            "QT": nc.dram_tensor("dbg_QT", [BPC, HD, T], BF16, kind="ExternalOutput"),
            "headsT": nc.dram_tensor("dbg_headsT", [BPC, HD, T], BF16, kind="ExternalOutput"),
            "h1T": nc.dram_tensor("dbg_h1T", [BPC, E, T], F32, kind="ExternalOutput"),
            "memT": nc.dram_tensor("dbg_memT", [BPC, E, T], F32, kind="ExternalOutput"),
            "decT": nc.dram_tensor("dbg_decT", [E, BPC], F32, kind="ExternalOutput"),
            "Lraw": nc.dram_tensor("dbg_Lraw", [BPC, T], F32, kind="ExternalOutput"),
        }

    import os
    phases = int(os.environ.get("KPHASES", "5"))
    krep = int(os.environ.get("KREP", "1"))
    with tile.TileContext(nc) as tc:
        for i in range(krep):
            with contextlib.ExitStack() as ctx:
                _emit(nc, tc, tens, dbg if i == 0 else None, ctx, phases=phases)
    nc.finalize()
    return nc


_built = {}


def _get_nc(debug=False):
    key = bool(debug)
    if key not in _built:
        _built[key] = build(debug=key)
    return _built[key]


def make_in_maps(inputs):
    in_maps = []
    for c in range(NCORES):
        s = slice(BPC * c, BPC * (c + 1))
        m = {
            "src": np.ascontiguousarray(inputs["src"][s]),
            "tgt": np.ascontiguousarray(inputs["tgt"][s]),
            "enc_mask": np.ascontiguousarray(inputs["enc_mask"][s]),
            "dec_mask": np.ascontiguousarray(inputs["dec_mask"][s]),
        }
        for nm in WEIGHT_NAMES:
            m[nm] = np.asarray(inputs[nm])
        in_maps.append(m)
    return in_maps


def kernel(**inputs):
    nc = _get_nc(debug=False)
    in_maps = make_in_maps(inputs)
    res = run_bass_kernel_spmd(nc, in_maps, list(range(NCORES)))
    out = np.concatenate([res.results[c]["out"] for c in range(NCORES)], axis=0)
    return out.astype(np.float32)


# revision 47
# speedup vs baseline: 6.3697x; 6.3697x over previous
"""Trainium2 Bass kernel for nn_AttentionNet (encoder layer + 1-step decoder + pointer).

Sharding: data-parallel over batch, 4 batches per core x 8 cores.
Layout strategy: everything kept transposed ([feature, token], feature on
partitions) so no on-chip transposes are needed anywhere; LN stats are
computed with ones/indicator-matmuls over the partition axis; softmax is
max-free (scores are small) with the mask applied multiplicatively after
exp; softmax denominators come from col-packed ones-matmuls.
"""

import math
import os

import numpy as np

MASKMODE = int(os.environ.get("KMASKMODE", "0"))
KATT = int(os.environ.get("KATT", "0"))

import concourse.bacc as bacc
import concourse.bass as bass
import concourse.tile as tile
from concourse import mybir
from concourse.bass_utils import run_bass_kernel_spmd

F32 = mybir.dt.float32
F32R = mybir.dt.float32r
BF16 = mybir.dt.bfloat16
I32 = mybir.dt.int32
AF = mybir.ActivationFunctionType
OP = mybir.AluOpType

E, H, D, FF = 128, 4, 32, 512
HD = H * D
B, T, Q = 32, 1024, 1
NCORES = 8
BPC = B // NCORES  # batches per core
NTK = T // 128     # tk tiles
TQC = 256          # tq chunk for attention
NTQ = T // TQC

WEIGHT_NAMES = [
    "enc_wq", "enc_wk", "enc_wv", "enc_wo", "enc_ln1_g", "enc_ln1_b",
    "enc_ln2_g", "enc_ln2_b", "enc_ffn_w1", "enc_ffn_b1", "enc_ffn_w2",
    "enc_ffn_b2",
    "dec_wq", "dec_wk", "dec_wv", "dec_wo", "dec_ln1_g", "dec_ln1_b",
    "dec_ln2_g", "dec_ln2_b", "dec_ffn_w1", "dec_ffn_b1", "dec_ffn_w2",
    "dec_ffn_b2",
    "ptr_wq", "ptr_wk",
]


def r(ap):
    """View an f32 AP as float32r for full-speed PE matmuls."""
    return ap.bitcast(F32R)


_EMIT_N = [0]


def _emit(nc, tc, tens, dbg, ctx, phases=5):
    rep = _EMIT_N[0]; _EMIT_N[0] += 1
    sc = 1.0 / math.sqrt(D)

    singles = ctx.enter_context(tc.tile_pool(name="singles", bufs=1))
    # One PSUM pool for the whole kernel: S-tag [128,1024] (2 banks) x3 +
    # acc-tag [128,256] (1 bank) x2 = 8 banks exactly.
    psum = ctx.enter_context(tc.tile_pool(name="psum", bufs=1, space="PSUM"))

    cnt = [0]

    def ps_big():
        cnt[0] += 1
        return psum.tile([128, 1024], F32, tag="S", name=f"psS{cnt[0]}", bufs=3)

    def ps_acc():
        cnt[0] += 1
        return psum.tile([128, TQC], F32, tag="acc", name=f"psA{cnt[0]}", bufs=2)

    # ---- weights to SBUF (f32 loads + bf16 casts) ----
    def load_w(shape, nm, in_ap, out_view=None):
        tl = singles.tile(shape, F32, tag=f"{nm}", name=f"{nm}")
        nc.sync.dma_start(out=tl[:] if out_view is None else out_view(tl), in_=in_ap)
        return tl

    def bfcast(tl, nm):
        tb = singles.tile(list(tl.shape), BF16, tag=f"{nm}b", name=f"{nm}b")
        nc.vector.tensor_copy(tb[:], tl[:])
        return tb

    wq_all, wk_all, wv_all, wo_all = {}, {}, {}, {}
    for pfx in ("enc", "dec"):
        for nm, store in (("wq", wq_all), ("wk", wk_all), ("wv", wv_all)):
            tl = load_w([E, HD], f"{pfx}_{nm}",
                        tens[f"{pfx}_{nm}"].ap().rearrange("h e d -> e h d"),
                        out_view=lambda t: t[:].rearrange("e (h d) -> e h d", h=H))
            store[pfx] = bfcast(tl, f"{pfx}_{nm}")
        tl = load_w([HD, E], f"{pfx}_wo", tens[f"{pfx}_wo"].ap().rearrange("h d e -> (h d) e"))
        wo_all[pfx] = bfcast(tl, f"{pfx}_wo")

    w1 = {}; w2bf = {}; b1t = {}; b2c = {}
    for pfx in ("enc", "dec"):
        tl = load_w([E, FF], f"{pfx}_w1", tens[f"{pfx}_ffn_w1"].ap())
        w1[pfx] = bfcast(tl, f"{pfx}_w1")
        tf = load_w([128, 4 * E], f"{pfx}_w2",
                    tens[f"{pfx}_ffn_w2"].ap().rearrange("(c p) e -> p c e", p=128),
                    out_view=lambda t: t[:].rearrange("p (c e) -> p c e", c=4))
        w2bf[pfx] = bfcast(tf, f"{pfx}_w2")
        b1t[pfx] = load_w([128, 4], f"{pfx}_b1",
                          tens[f"{pfx}_ffn_b1"].ap().rearrange("(c p) -> p c", p=128))
        b2c[pfx] = load_w([128, 1], f"{pfx}_b2",
                          tens[f"{pfx}_ffn_b2"].ap().rearrange("(e q) -> e q", q=1))

    ptrq = load_w([E, E], "ptr_wq", tens["ptr_wq"].ap())
    ptrk = load_w([E, E], "ptr_wk", tens["ptr_wk"].ap())

    # LN gain/bias rows, each its own [1, E] tile at partition 0 (f32 + bf16)
    ln_rows = {}
    for nm in ["enc_ln1_g", "enc_ln1_b", "enc_ln2_g", "enc_ln2_b",
               "dec_ln1_g", "dec_ln1_b", "dec_ln2_g", "dec_ln2_b"]:
        row = load_w([1, E], f"row_{nm}", tens[nm].ap().rearrange("(q e) -> q e", q=1))
        ln_rows[nm] = bfcast(row, f"row_{nm}")

    # ---- constants (inline numpy, DMA to SBUF) ----
    def const_tile(arr, dt, nm):
        arr = np.asarray(arr)
        if dt == BF16:
            import ml_dtypes
            arr = arr.astype(ml_dtypes.bfloat16)
        elif dt == F32:
            arr = arr.astype(np.float32)
        else:
            arr = arr.astype(np.int32)
        h = nc.inline_tensor(arr, name=f"c{rep}_{nm}")
        tl = singles.tile(list(arr.shape), dt, tag=f"c_{nm}", name=f"ct_{nm}")
        nc.sync.dma_start(out=tl[:], in_=h.ap())
        return tl

    ones_col_bf = const_tile(np.ones((128, 1)), BF16, "ones_col_bf")
    ones_row = const_tile(np.ones((1, 512)), BF16, "ones_row")
    vc = np.ones((1, 128)); vc[0, ::32] = 0.0
    vcomp_row = const_tile(vc, BF16, "vcomp_row")
    a = np.zeros((128, 128))
    for h in range(H):
        a[32 * h, 32 * h:32 * (h + 1)] = 1.0
    e4sel = const_tile(a, F32, "e4sel")
    # ind8: 8 blocks of width 36: x-stats of batch b -> row b; sq-stats -> row 32+b
    a = np.zeros((128, 8 * 36))
    for b in range(4):
        a[:, 36 * b + b] = 1.0
        a[:, 36 * (4 + b) + 32 + b] = 1.0
    ind8 = const_tile(a, BF16, "ind8")
    a = np.zeros((32, 4))
    for i in range(32):
        a[i, i % 4] = 1.0
    p32 = const_tile(a, F32, "p32")
    a = np.zeros((4, 128))
    for h in range(H):
        a[h, 32 * h:32 * (h + 1)] = 1.0
    e4t = const_tile(a, F32, "e4t")
    eps4 = const_tile(np.full((4, 1), 1e-5), F32, "eps4")
    eps1 = const_tile(np.full((1, 1), 1e-5), F32, "eps1")
    # dsel[k, 4b+h] = 1 iff k == b
    a = np.zeros((4, 16))
    for b in range(BPC):
        a[b, 4 * b:4 * (b + 1)] = 1.0
    dsel = const_tile(a, BF16, "dsel")
    # ind2: [:, 0:33] col0 ones (sum -> row 0); [:, 33:66] col32 ones (sumsq -> row 32)
    a = np.zeros((128, 66))
    a[:, 0] = 1.0
    a[:, 33 + 32] = 1.0
    ind2 = const_tile(a, BF16, "ind2")
    ident = const_tile(np.eye(128), BF16, "ident")
    # eb4[0, 4b:4b+4] has 1 at col b (for building gsel rows via K=1 matmuls)
    a = np.zeros((1, 16))
    for b in range(BPC):
        a[0, 4 * b + b] = 1.0
    eb4 = const_tile(a, BF16, "eb4")

    big = ctx.enter_context(tc.tile_pool(name="big", bufs=1))
    scratch = ctx.enter_context(tc.tile_pool(name="scratch", bufs=2))
    sc1 = ctx.enter_context(tc.tile_pool(name="sc1", bufs=1))

    # ---- load srcT ----
    srcT = []
    for b in range(BPC):
        tl = big.tile([E, T], F32, tag=f"srcT{b}")
        nc.sync.dma_start(out=tl[:], in_=tens["src"].ap()[b].rearrange("t e -> e t"))
        srcT.append(tl)

    def ln_group(xs, g_row, b_row):
        """LayerNorm over the partition axis (E) for BPC transposed [128, T] f32 tiles.

        Stats via indicator-matmuls (bf16 inputs), apply via outer-product
        broadcast matmuls. Returns apply(b, out_tile) emitting bf16 output."""
        stats_ps = ps_big()  # rows 0..3 sums, 32..35 sumsq
        nb = len(xs)
        for b, xt in enumerate(xs):
            xbf = sc1.tile([128, T], BF16, tag="lnxbf", name=f"lnxbf{b}", bufs=2)
            nc.vector.tensor_copy(xbf[:], xt[:])
            sq = sc1.tile([128, T], BF16, tag="lnsq", name=f"lnsq{b}", bufs=2)
            nc.gpsimd.tensor_mul(sq[:], xbf[:], xbf[:])
            for c in range(2):
                s = slice(512 * c, 512 * (c + 1))
                nc.tensor.matmul(stats_ps[0:36, s], ind8[:, 36 * b:36 * (b + 1)], xbf[:, s],
                                 start=(b == 0), stop=False)
                nc.tensor.matmul(stats_ps[0:36, s], ind8[:, 36 * (4 + b):36 * (5 + b)], sq[:, s],
                                 start=False, stop=(b == nb - 1))
        stats = scratch.tile([36, T], F32, tag="lnst", bufs=1)
        nc.vector.tensor_copy(stats[:], stats_ps[0:36, :])
        m = scratch.tile([4, T], F32, tag="lnm", bufs=1)
        nc.vector.tensor_scalar_mul(m[0:nb], stats[0:nb], 1.0 / E)
        var = scratch.tile([4, T], F32, tag="lnv", bufs=1)
        nc.vector.tensor_scalar_mul(var[0:nb], stats[32:32 + nb], 1.0 / E)
        msq = scratch.tile([4, T], F32, tag="lnmsq", bufs=1)
        nc.vector.tensor_mul(msq[0:nb], m[0:nb], m[0:nb])
        nc.vector.tensor_sub(var[0:nb], var[0:nb], msq[0:nb])
        nc.scalar.activation(out=var[0:nb], in_=var[0:nb], func=AF.Ln, bias=eps4[0:nb, 0:1])
        rs = scratch.tile([4, T], BF16, tag="lnrs", bufs=1)
        nc.scalar.activation(out=rs[0:nb], in_=var[0:nb], func=AF.Exp, scale=-0.5)
        nmrs = scratch.tile([4, T], BF16, tag="lnnm", bufs=1)
        nc.vector.scalar_tensor_tensor(out=nmrs[0:nb], in0=m[0:nb], scalar=-1.0,
                                       in1=rs[0:nb], op0=OP.mult, op1=OP.mult)
        # gsel[0:4, 128b:128(b+1)] has g in row b, zeros elsewhere
        gsel_ps = ps_big()
        for b in range(nb):
            nc.tensor.matmul(gsel_ps[0:4, E * b:E * (b + 1)], eb4[:, 4 * b:4 * (b + 1)],
                             g_row[:], start=True, stop=True)
        gsel = scratch.tile([4, 4 * E], BF16, tag="gsel", bufs=1)
        nc.vector.tensor_copy(gsel[:], gsel_ps[0:4, 0:4 * E])

        def apply(b, ot):
            xt = xs[b]
            for c in range(2):
                s = slice(512 * c, 512 * (c + 1))
                a_ps = ps_big()
                nc.tensor.matmul(a_ps[:, :512], gsel[:, E * b:E * (b + 1)], rs[0:4, s],
                                 start=True, stop=True)
                b_ps = ps_big()
                nc.tensor.matmul(b_ps[:, :512], gsel[:, E * b:E * (b + 1)], nmrs[0:4, s],
                                 start=True, stop=False)
                nc.tensor.matmul(b_ps[:, :512], b_row[:], ones_row[:], start=False, stop=True)
                tmp = scratch.tile([128, 512], F32, tag="lntmp")
                nc.vector.scalar_tensor_tensor(out=tmp[:], in0=xt[:, s], scalar=1.0,
                                               in1=a_ps[:, :512], op0=OP.bypass, op1=OP.mult)
                nc.vector.scalar_tensor_tensor(out=ot[:, s], in0=tmp[:], scalar=1.0,
                                               in1=b_ps[:, :512], op0=OP.bypass, op1=OP.add)
            return ot
        return apply

    def mk_ln_out(nm):
        return scratch.tile([E, T], BF16, tag="xln", name=nm, bufs=1)

    # ---- encoder LN1 ----
    with nc.named_scope("enc_ln1"):
        apply_ln1 = ln_group(srcT, ln_rows["enc_ln1_g"], ln_rows["enc_ln1_b"])

    # ---- per-batch: QKV + attention ----
    h1T = []
    with nc.named_scope("enc_attn"):
        for b in range(BPC):
            xt = apply_ln1(b, mk_ln_out(f"xln{b}"))
            qhat = scratch.tile([HD, H * T], BF16, tag="qhat")
            nc.vector.memset(qhat[:], 0.0)
            kt = scratch.tile([HD, T], BF16, tag="KT")
            vn = scratch.tile([128, NTK * HD], BF16, tag="Vn")
            for c in range(2):
                s = slice(512 * c, 512 * (c + 1))
                ps = ps_big()
                nc.tensor.matmul(ps[:, :512], wq_all["enc"][:], xt[:, s], start=True, stop=True)
                for h in range(H):
                    dst = qhat[32 * h:32 * (h + 1), :].rearrange(
                        "p (c2 h2 t) -> p c2 h2 t", c2=NTQ, h2=H)[:, 2 * c:2 * c + 2, h, :]
                    src = ps[32 * h:32 * (h + 1), 0:512].rearrange("p (c2 t) -> p c2 t", c2=2)
                    nc.vector.tensor_copy(dst, src)
                ps2 = ps_big()
                nc.tensor.matmul(ps2[:, :512], wk_all["enc"][:], xt[:, s], start=True, stop=True)
                nc.vector.tensor_copy(kt[:, s], ps2[:, :512])
            vps = ps_big()
            for c in range(NTK):
                nc.tensor.matmul(vps[:, HD * c:HD * (c + 1)],
                                 xt[:, 128 * c:128 * (c + 1)], wv_all["enc"][:],
                                 start=True, stop=True)
            nc.vector.tensor_copy(vn[:], vps[:])
            if dbg is not None:
                nc.sync.dma_start(out=dbg["xlnT"].ap()[b], in_=xt[:])

            if phases < 2:
                nc.gpsimd.dma_start(out=tens["out"].ap().rearrange("b q t -> (b q) t"),
                                    in_=qt[0:BPC, :])
                return
            h1 = big.tile([E, T], F32, tag=f"h1T{b}")
            # natural-orientation mask tiles (full-BW DMA), cast to (1-m) bf16,
            # then PE-transpose 128x128 blocks: wt[k][:, 128j:] = wn[j][:, 128k:]^T
            wt = [scratch.tile([128, T], BF16, tag=f"w{k}", bufs=1, name=f"w{k}_{b}")
                  for k in range(NTK)]
            for half in range(2):
                wn = []
                for j4 in range(4):
                    j = 4 * half + j4
                    mi = scratch.tile([128, T], I32, tag="mi", bufs=2)
                    nc.sync.dma_start(out=mi[:],
                                      in_=tens["enc_mask"].ap()[b][128 * j:128 * (j + 1), :])
                    wj = scratch.tile([128, T], BF16, tag=f"wn{j4}", bufs=1, name=f"wn{j4}_{half}")
                    nc.vector.tensor_scalar(out=wj[:], in0=mi[:], scalar1=-1.0, scalar2=1.0,
                                            op0=OP.mult, op1=OP.add)
                    wn.append(wj)
                for k in range(NTK):
                    cnt[0] += 1
                    tp = psum.tile([128, 512], BF16, tag="acc", name=f"psT{cnt[0]}", bufs=2)
                    for j4 in range(4):
                        nc.tensor.matmul(tp[:, 128 * j4:128 * (j4 + 1)],
                                         wn[j4][:, 128 * k:128 * (k + 1)], ident[:],
                                         start=True, stop=True, is_transpose=True)
                    nc.vector.tensor_copy(wt[k][:, 512 * half:512 * (half + 1)], tp[:])
            if KATT == 1:
                for k in range(NTK):
                    nc.vector.tensor_copy(h1[:, 128 * k:128 * (k + 1)],
                                          wt[k][:, 0:128])
                h1T.append(h1)
                continue
            for tq in range(NTQ):
                qs = slice(TQC * tq, TQC * (tq + 1))
                vacc = ps_acc()
                dacc = ps_acc()
                nc.tensor.matmul(dacc[:], vcomp_row[:], ones_row[:, :TQC], start=True, stop=False)
                for k in range(NTK):
                    S = ps_big()
                    for half in range(2):
                        fs = slice(512 * half, 512 * (half + 1))
                        nc.tensor.matmul(S[:, fs], kt[:, 128 * k:128 * (k + 1)],
                                         qhat[:, H * TQC * tq + 512 * half:
                                              H * TQC * tq + 512 * (half + 1)],
                                         start=True, stop=True)
                    eu = scratch.tile([128, H * TQC], BF16, tag="eu")
                    nc.scalar.activation(out=eu[:], in_=S[:], func=AF.Exp, scale=sc)
                    ws = wt[k][:, qs]
                    if MASKMODE == 0:
                        w_rep = bass.AP(tensor=ws.tensor, offset=ws.offset,
                                        ap=[ws.ap[0], [0, H], ws.ap[1]])
                        eu3 = eu[:].rearrange("p (h t) -> p h t", h=H)
                        nc.gpsimd.tensor_tensor(out=eu3, in0=eu3, in1=w_rep, op=OP.mult)
                    elif MASKMODE == 1:
                        for h in range(H):
                            es0 = eu[:, TQC * h:TQC * (h + 1)]
                            nc.vector.tensor_mul(es0, es0, ws)
                    else:
                        for h in range(H):
                            es0 = eu[:, TQC * h:TQC * (h + 1)]
                            nc.gpsimd.tensor_mul(es0, es0, ws)
                    for h in range(H):
                        es = eu[:, TQC * h:TQC * (h + 1)]
                        nc.tensor.matmul(vacc[32 * h:32 * (h + 1), :],
                                         vn[:, HD * k + 32 * h:HD * k + 32 * (h + 1)],
                                         es, start=(k == 0), stop=(k == NTK - 1),
                                         tile_position=(0, 32 * h))
                        nc.tensor.matmul(dacc[32 * h:32 * h + 1, :],
                                         ones_col_bf[:], es,
                                         start=False, stop=(k == NTK - 1 and h == H - 1),
                                         tile_position=(0, 32 * h))
                recip = scratch.tile([128, TQC], F32, tag="recip")
                nc.vector.reciprocal(recip[:], dacc[:])
                rb_ps = ps_acc()
                nc.tensor.matmul(rb_ps[:], e4sel[:], recip[:], start=True, stop=True)
                rb = scratch.tile([128, TQC], F32, tag="rbs")
                nc.vector.tensor_copy(rb[:], rb_ps[:])
                hn = scratch.tile([128, TQC], BF16, tag="hn")
                nc.vector.tensor_mul(hn[:], vacc[:], rb[:])
                at_ps = ps_acc()
                nc.tensor.matmul(at_ps[:], wo_all["enc"][:], hn[:], start=True, stop=True)
                nc.vector.tensor_add(h1[:, qs], at_ps[:], srcT[b][:, qs])
                if dbg is not None:
                    nc.sync.dma_start(out=dbg["headsT"].ap()[b][:, qs], in_=hn[:])
            if KATT == 2:
                h1T.append(h1)
                continue
            h1T.append(h1)
            if dbg is not None:
                nc.sync.dma_start(out=dbg["h1T"].ap()[b], in_=h1[:])

    if phases < 3:
        nc.sync.dma_start(out=tens["out"].ap().rearrange("b q t -> (b q) t"),
                          in_=h1T[0][0:BPC, :])
        return

    # ---- encoder LN2 + FFN ----
    with nc.named_scope("enc_ffn"):
        apply_ln2 = ln_group(h1T, ln_rows["enc_ln2_g"], ln_rows["enc_ln2_b"])
        memT = []
        for b in range(BPC):
            hlnb = apply_ln2(b, mk_ln_out(f"hln{b}"))
            mt = big.tile([E, T], F32, tag=f"memT{b}")
            act1 = scratch.tile([128, 4 * T], BF16, tag="act1", bufs=1)
            for fc in range(4):
                for c in range(2):
                    s = slice(512 * c, 512 * (c + 1))
                    ps = ps_big()
                    nc.tensor.matmul(ps[:, :512], w1["enc"][:, 128 * fc:128 * (fc + 1)],
                                     hlnb[:, s], start=True, stop=True)
                    nc.scalar.activation(out=act1[:, T * fc + 512 * c:T * fc + 512 * (c + 1)],
                                         in_=ps[:, :512], func=AF.Relu,
                                         bias=b1t["enc"][:, fc:fc + 1], scale=1.0)
            for c in range(2):
                s = slice(512 * c, 512 * (c + 1))
                ps = ps_big()
                for fc in range(4):
                    nc.tensor.matmul(ps[:, :512], w2bf["enc"][:, 128 * fc:128 * (fc + 1)],
                                     act1[:, T * fc + 512 * c:T * fc + 512 * (c + 1)],
                                     start=(fc == 0), stop=(fc == 3))
                nc.vector.scalar_tensor_tensor(out=mt[:, s], in0=ps[:, :512],
                                               scalar=b2c["enc"][:, 0:1],
                                               in1=h1T[b][:, s], op0=OP.add, op1=OP.add)
            memT.append(mt)
            if dbg is not None:
                nc.sync.dma_start(out=dbg["memT"].ap()[b], in_=mt[:])

    if phases < 4:
        nc.sync.dma_start(out=tens["out"].ap().rearrange("b q t -> (b q) t"),
                          in_=memT[0][0:BPC, :])
        return

    # ---- decoder ----
    with nc.named_scope("decoder"):
        tgtT = singles.tile([E, BPC], F32, tag="tgtT")
        nc.sync.dma_start(out=tgtT[:], in_=tens["tgt"].ap().rearrange("b q e -> e (b q)"))
        dmi = scratch.tile([BPC, T], I32, tag="mi", name="dmi", bufs=2)
        nc.sync.dma_start(out=dmi[:], in_=tens["dec_mask"].ap().rearrange("b q t -> (b q) t"))
        dmf = singles.tile([BPC, T], BF16, tag="dmf")  # -10000 * mask (decoder attn)
        nc.vector.tensor_scalar_mul(dmf[:], dmi[:], -10000.0)
        dmf32 = singles.tile([BPC, T], F32, tag="dmf32")  # exact, for pointer logits
        nc.vector.tensor_scalar_mul(dmf32[:], dmi[:], -10000.0)
        dwf = singles.tile([BPC, T], F32, tag="dwf")  # 1 - mask
        nc.vector.tensor_scalar(out=dwf[:], in0=dmi[:], scalar1=-1.0, scalar2=1.0,
                                op0=OP.mult, op1=OP.add)

        apply_lnm = ln_group(memT, ln_rows["dec_ln1_g"], ln_rows["dec_ln1_b"])

        def ln_small(x, n, g_row, b_row, tagn):
            """LN over partitions for a small [E, n] f32 tile -> bf16 out."""
            xbf = scratch.tile([E, BPC], BF16, tag=f"dxb{tagn}")
            nc.vector.tensor_copy(xbf[:, :n], x[:, :n])
            sq = scratch.tile([E, BPC], BF16, tag=f"dsq{tagn}")
            nc.gpsimd.tensor_mul(sq[:, :n], xbf[:, :n], xbf[:, :n])
            st_ps = ps_big()  # row 0 (sum), row 32 (sumsq) via ind2
            nc.tensor.matmul(st_ps[0:33, :n], ind2[:, 0:33], xbf[:, :n], start=True, stop=False)
            nc.tensor.matmul(st_ps[0:33, :n], ind2[:, 33:66], sq[:, :n], start=False, stop=True)
            st = scratch.tile([33, BPC], F32, tag=f"dss{tagn}")
            nc.vector.tensor_copy(st[:, :n], st_ps[0:33, :n])
            mn = scratch.tile([1, BPC], F32, tag=f"dmn{tagn}")
            nc.vector.tensor_scalar_mul(mn[:, :n], st[0:1, :n], 1.0 / E)
            msq = scratch.tile([1, BPC], F32, tag=f"dmsq{tagn}")
            nc.vector.tensor_mul(msq[:, :n], mn[:, :n], mn[:, :n])
            var = scratch.tile([1, BPC], F32, tag=f"dvar{tagn}")
            nc.vector.tensor_scalar(out=var[:, :n], in0=st[32:33, :n], scalar1=1.0 / E,
                                    scalar2=None, op0=OP.mult)
            nc.vector.tensor_sub(var[:, :n], var[:, :n], msq[:, :n])
            nc.scalar.activation(out=var[:, :n], in_=var[:, :n], func=AF.Ln, bias=eps1[:])
            rs = scratch.tile([1, BPC], BF16, tag=f"drs{tagn}")
            nc.scalar.activation(out=rs[:, :n], in_=var[:, :n], func=AF.Exp, scale=-0.5)
            nmrs = scratch.tile([1, BPC], BF16, tag=f"dnm{tagn}")
            nc.vector.scalar_tensor_tensor(out=nmrs[:, :n], in0=mn[:, :n], scalar=-1.0,
                                           in1=rs[:, :n], op0=OP.mult, op1=OP.mult)
            a_ps = ps_acc()
            nc.tensor.matmul(a_ps[:, :n], g_row[:], rs[:, :n], start=True, stop=True)
            b_ps = ps_acc()
            nc.tensor.matmul(b_ps[:, :n], g_row[:], nmrs[:, :n], start=True, stop=False)
            nc.tensor.matmul(b_ps[:, :n], b_row[:], ones_row[:, :n], start=False, stop=True)
            tmp = scratch.tile([E, BPC], F32, tag=f"dtmp{tagn}")
            nc.vector.scalar_tensor_tensor(out=tmp[:, :n], in0=x[:, :n], scalar=1.0,
                                           in1=a_ps[:, :n], op0=OP.bypass, op1=OP.mult)
            out = scratch.tile([E, BPC], BF16, tag=f"dout{tagn}")
            nc.vector.scalar_tensor_tensor(out=out[:, :n], in0=tmp[:, :n], scalar=1.0,
                                           in1=b_ps[:, :n], op0=OP.bypass, op1=OP.add)
            return out

        tln = ln_small(tgtT, BPC, ln_rows["dec_ln1_g"], ln_rows["dec_ln1_b"], "t")
        qd_ps = ps_acc()
        nc.tensor.matmul(qd_ps[0:HD, 0:BPC], wq_all["dec"][:], tln[:, :BPC], start=True, stop=True)
        qdec = scratch.tile([HD, BPC], BF16, tag="qdec")
        nc.vector.tensor_copy(qdec[:], qd_ps[0:HD, 0:BPC])

        h1d = singles.tile([E, BPC], F32, tag="h1d")
        for b in range(BPC):
            mlnb = apply_lnm(b, mk_ln_out(f"mln{b}"))
            kd = scratch.tile([HD, T], BF16, tag="kdec", bufs=1)
            vd = scratch.tile([128, NTK * HD], BF16, tag="vdec", bufs=1)
            for c in range(2):
                s = slice(512 * c, 512 * (c + 1))
                ps = ps_big()
                nc.tensor.matmul(ps[:, :512], wk_all["dec"][:], mlnb[:, s], start=True, stop=True)
                nc.vector.tensor_copy(kd[:, s], ps[:, :512])
            vps = ps_big()
            for c in range(NTK):
                nc.tensor.matmul(vps[:, HD * c:HD * (c + 1)],
                                 mlnb[:, 128 * c:128 * (c + 1)], wv_all["dec"][:],
                                 start=True, stop=True)
            nc.vector.tensor_copy(vd[:], vps[:])
            qblk = scratch.tile([HD, 4], BF16, tag="qblk")
            nc.vector.memset(qblk[:], 0.0)
            for h in range(H):
                nc.vector.tensor_copy(qblk[32 * h:32 * (h + 1), h:h + 1],
                                      qdec[32 * h:32 * (h + 1), b:b + 1])
            ud_ps = ps_acc()
            for k in range(NTK):
                cs = slice(4 * k, 4 * (k + 1))
                nc.tensor.matmul(ud_ps[:, cs], kd[:, 128 * k:128 * (k + 1)], qblk[:],
                                 start=True, stop=False)
                nc.tensor.matmul(ud_ps[:, cs], dmf[0:4, 128 * k:128 * (k + 1)],
                                 dsel[:, 4 * b:4 * (b + 1)], start=False, stop=True)
            eud = scratch.tile([128, 4 * NTK], BF16, tag="eud")
            nc.scalar.activation(out=eud[:], in_=ud_ps[:, 0:4 * NTK], func=AF.Exp, scale=sc)
            d1_ps = ps_acc()
            nc.tensor.matmul(d1_ps[0:32, 0:1], eud[:], ones_col_bf[:], start=True, stop=True)
            d1 = scratch.tile([32, 1], F32, tag="d1s")
            nc.vector.tensor_copy(d1[:], d1_ps[0:32, 0:1])
            d4_ps = ps_acc()
            nc.tensor.matmul(d4_ps[0:4, 0:1], p32[:], d1[:], start=True, stop=True)
            rc4 = scratch.tile([4, 1], F32, tag="rc4")
            nc.vector.reciprocal(rc4[:], d4_ps[0:4, 0:1])
            rb_ps = ps_acc()
            nc.tensor.matmul(rb_ps[:, 0:1], e4t[:], rc4[:], start=True, stop=True)
            rb = scratch.tile([128, 1], F32, tag="rb128s")
            nc.vector.tensor_copy(rb[:], rb_ps[:, 0:1])
            hd_ps = ps_acc()
            for k in range(NTK):
                nc.tensor.matmul(hd_ps[:, 0:4], vd[:, HD * k:HD * (k + 1)], eud[:, 4 * k:4 * (k + 1)],
                                 start=(k == 0), stop=(k == NTK - 1))
            hdec = scratch.tile([HD, 1], BF16, tag="hdec")
            for h in range(H):
                nc.vector.tensor_copy(hdec[32 * h:32 * (h + 1), 0:1],
                                      hd_ps[32 * h:32 * (h + 1), h:h + 1])
            nc.vector.tensor_scalar_mul(hdec[:], hdec[:], rb[:, 0:1])
            ao_ps = ps_acc()
            nc.tensor.matmul(ao_ps[:, 0:1], wo_all["dec"][:], hdec[:], start=True, stop=True)
            nc.vector.tensor_add(h1d[:, b:b + 1], ao_ps[:, 0:1], tgtT[:, b:b + 1])

        hln2d = ln_small(h1d, BPC, ln_rows["dec_ln2_g"], ln_rows["dec_ln2_b"], "d2")
        dact_ps = ps_acc()
        for fc in range(4):
            nc.tensor.matmul(dact_ps[:, 4 * fc:4 * (fc + 1)],
                             w1["dec"][:, 128 * fc:128 * (fc + 1)], hln2d[:, :BPC],
                             start=True, stop=True)
        dact = scratch.tile([128, 16], BF16, tag="dacts")
        for fc in range(4):
            nc.scalar.activation(out=dact[:, 4 * fc:4 * (fc + 1)],
                                 in_=dact_ps[:, 4 * fc:4 * (fc + 1)], func=AF.Relu,
                                 bias=b1t["dec"][:, fc:fc + 1], scale=1.0)
        do_ps = ps_acc()
        for fc in range(4):
            nc.tensor.matmul(do_ps[:, 0:BPC], w2bf["dec"][:, 128 * fc:128 * (fc + 1)],
                             dact[:, 4 * fc:4 * (fc + 1)],
                             start=(fc == 0), stop=(fc == 3))
        decT = singles.tile([E, BPC], F32, tag="decT")
        tmp2 = scratch.tile([E, BPC], F32, tag="dtmp2")
        nc.vector.tensor_scalar_add(tmp2[:], do_ps[:, 0:BPC], b2c["dec"][:, 0:1])
        nc.vector.tensor_add(decT[:], tmp2[:], h1d[:])
        if dbg is not None:
            nc.sync.dma_start(out=dbg["decT"].ap(), in_=decT[:])

    if phases < 5:
        nc.sync.dma_start(out=tens["out"].ap().rearrange("b q t -> (b q) t"),
                          in_=h1d[0:BPC, 0:T] if False else memT[0][0:BPC, :])
        return

    # ---- pointer (f32 matmuls for accuracy) ----
    with nc.named_scope("pointer"):
        qp_ps = ps_acc()
        nc.tensor.matmul(qp_ps[:, 0:BPC], ptrq[:], decT[:], start=True, stop=True)
        qpi = scratch.tile([E, 4 * BPC], F32, tag="qpi")
        nc.vector.memset(qpi[:], 0.0)
        for b in range(BPC):
            nc.vector.tensor_copy(qpi[:, 5 * b:5 * b + 1], qp_ps[:, b:b + 1])
        up_ps = ps_big()
        for b in range(BPC):
            kp = scratch.tile([E, T], F32, tag="kdec", name="kps", bufs=1)
            for c in range(2):
                s = slice(512 * c, 512 * (c + 1))
                kpc = ps_big()
                nc.tensor.matmul(kpc[:, :512], ptrk[:], memT[b][:, s], start=True, stop=True)
                nc.vector.tensor_copy(kp[:, s], kpc[:, :512])
            for c in range(2):
                s = slice(512 * c, 512 * (c + 1))
                nc.tensor.matmul(up_ps[0:BPC, s], qpi[:, 4 * b:4 * (b + 1)], kp[:, s],
                                 start=(b == 0), stop=(b == BPC - 1))
        # L = 10*tanh(U/sqrt(E)) ; tanh(x) = 1 - 2/(exp(2x)+1)
        e2 = scratch.tile([BPC, T], F32, tag="lnm", name="pe2", bufs=1)
        nc.scalar.activation(out=e2[:], in_=up_ps[0:BPC, :], func=AF.Exp, scale=2.0 / math.sqrt(E))
        nc.vector.tensor_scalar_add(e2[:], e2[:], 1.0)
        rec = scratch.tile([BPC, T], F32, tag="lnv", name="prec", bufs=1)
        nc.vector.reciprocal(rec[:], e2[:])
        L = scratch.tile([BPC, T], F32, tag="lnmsq", name="pL", bufs=1)
        nc.vector.tensor_scalar(out=L[:], in0=rec[:], scalar1=-20.0, scalar2=10.0,
                                op0=OP.mult, op1=OP.add)
        if dbg is not None:
            nc.sync.dma_start(out=dbg["Lraw"].ap(), in_=L[:])
        nc.vector.tensor_mul(L[:], L[:], dwf[:])
        nc.vector.tensor_add(L[:], L[:], dmf32[:])
        et = scratch.tile([BPC, T], F32, tag="lnst", name="pet", bufs=1)
        se = scratch.tile([BPC, 1], F32, tag="se")
        nc.scalar.activation(out=et[:], in_=L[:], func=AF.Exp, accum_out=se[:])
        lse = scratch.tile([BPC, 1], F32, tag="lse")
        nc.scalar.activation(out=lse[:], in_=se[:], func=AF.Ln)
        res = scratch.tile([BPC, T], F32, tag="mi", name="pres", bufs=2)
        nc.vector.tensor_scalar(out=res[:], in0=L[:], scalar1=lse[:, 0:1], scalar2=None,
                                op0=OP.subtract)
        nc.sync.dma_start(out=tens["out"].ap().rearrange("b q t -> (b q) t"), in_=res[:])


def build(debug=False):
    import contextlib
    nc = bacc.Bacc()
    tens = {}
    tens["src"] = nc.dram_tensor("src", [BPC, T, E], F32, kind="ExternalInput")
    tens["tgt"] = nc.dram_tensor("tgt", [BPC, Q, E], F32, kind="ExternalInput")
    tens["enc_mask"] = nc.dram_tensor("enc_mask", [BPC, T, T], I32, kind="ExternalInput")
    tens["dec_mask"] = nc.dram_tensor("dec_mask", [BPC, Q, T], I32, kind="ExternalInput")
    shapes = {
        "wq": [H, E, D], "wk": [H, E, D], "wv": [H, E, D], "wo": [H, D, E],
        "ln1_g": [E], "ln1_b": [E], "ln2_g": [E], "ln2_b": [E],
        "ffn_w1": [E, FF], "ffn_b1": [FF], "ffn_w2": [FF, E], "ffn_b2": [E],
    }
    for pfx in ("enc", "dec"):
        for nm, shp in shapes.items():
            full = f"{pfx}_{nm}"
            tens[full] = nc.dram_tensor(full, shp, F32, kind="ExternalInput")
    tens["ptr_wq"] = nc.dram_tensor("ptr_wq", [E, E], F32, kind="ExternalInput")
    tens["ptr_wk"] = nc.dram_tensor("ptr_wk", [E, E], F32, kind="ExternalInput")
    tens["out"] = nc.dram_tensor("out", [BPC, Q, T], F32, kind="ExternalOutput")

    dbg = None
    if debug:
        dbg = {
            "xlnT": nc.dram_tensor("dbg_xlnT", [BPC, E, T], BF16, kind="ExternalOutput"),
            "QT": nc.dram_tensor("dbg_QT", [BPC, HD, T], BF16, kind="ExternalOutput"),
            "headsT": nc.dram_tensor("dbg_headsT", [BPC, HD, T], BF16, kind="ExternalOutput"),
            "h1T": nc.dram_tensor("dbg_h1T", [BPC, E, T], F32, kind="ExternalOutput"),
            "memT": nc.dram_tensor("dbg_memT", [BPC, E, T], F32, kind="ExternalOutput"),
            "decT": nc.dram_tensor("dbg_decT", [E, BPC], F32, kind="ExternalOutput"),
            "Lraw": nc.dram_tensor("dbg_Lraw", [BPC, T], F32, kind="ExternalOutput"),
        }

    import os
    phases = int(os.environ.get("KPHASES", "5"))
    krep = int(os.environ.get("KREP", "1"))
    with tile.TileContext(nc) as tc:
        for i in range(krep):
            with contextlib.ExitStack() as ctx:
                _emit(nc, tc, tens, dbg if i == 0 else None, ctx, phases=phases)
    nc.finalize()
    return nc


_built = {}


def _get_nc(debug=False):
    key = bool(debug)
    if key not in _built:
        _built[key] = build(debug=key)
    return _built[key]


def make_in_maps(inputs):
    in_maps = []
    for c in range(NCORES):
        s = slice(BPC * c, BPC * (c + 1))
        m = {
            "src": np.ascontiguousarray(inputs["src"][s]),
            "tgt": np.ascontiguousarray(inputs["tgt"][s]),
            "enc_mask": np.ascontiguousarray(inputs["enc_mask"][s]),
            "dec_mask": np.ascontiguousarray(inputs["dec_mask"][s]),
        }
        for nm in WEIGHT_NAMES:
            m[nm] = np.asarray(inputs[nm])
        in_maps.append(m)
    return in_maps


def kernel(**inputs):
    nc = _get_nc(debug=False)
    in_maps = make_in_maps(inputs)
    res = run_bass_kernel_spmd(nc, in_maps, list(range(NCORES)))
    out = np.concatenate([res.results[c]["out"] for c in range(NCORES)], axis=0)
    return out.astype(np.float32)
